# revision 33
# baseline (speedup 1.0000x reference)
"""Adaptive Wing loss on 8 TRN2 NeuronCores (raw Bass, software-pipelined).

Inputs: input, target [64, 512, 512] f32. Output: scalar f32 sum.

Math (W=14, alpha=2.1, theta=0.5, eps=1): with d = |t - x|, e = 2.1 - t,
dc = min(d, 0.5), r = relu(d - 0.5), q = dc^e = exp(e*ln dc):

    loss/14 = log1p(q) + h(e)*r,   h(e) = 2e/(1+2^e)

(the wing branch d >= 0.5 is the first-order Taylor extension of
14*log1p(d^e) past 0.5; there sigma(e*ln dc) = 1/(1+2^e) exactly, and
r = 0 elsewhere so the wing term vanishes off-wing automatically).
h(e) is smooth on (1.1, 2.1]; wing elements only occur for t near 0 or
1 (weight (2t-1)^2/8), so a weighted linear fit h ~= C1*e + C0 adds
< 1e-3 relative error.

Measured TRN2 op rates (ns/elem, bf16 SBUF): TT 2x 0.54; plain TS 4x
0.30; TS f32-src 2x 0.58; any DVE accum variant (TENSOR_SCALAR_CACHE_
REDUCE / SCALAR_TENSOR_TENSOR) 1x 1.08; ACT pass (N+352)/1.2; tensor_
tensor_reduce does not compile (ISA wrong length), and matmuls with a
[1, N] psum output crash the exec unit (use full [128, N]).  So the
wing sum does NOT use a DVE accumulator: wt = r*h is a plain 2x TT and
the idle TensorE reduces it -- ones[128,128] @ wt chunk -> psum
[128, 512] per bank (all output partitions hold the same column sums),
accumulated across tiles (start/stop); partition 0 of the psum row is
copied to SBUF in two overlapped halves and summed on the host.

Engines (measured, per core): DVE 63.5us (diff TT f32 1x; d AND 4x;
dc min 4x; r relu 4x; m = eb*lnc TT 2x; h = C1*eb+C0 TS 4x; wt TT 2x;
eb TS f32 2x on EB_ON_DVE tiles), ACT 62.7us (Ln dc, Exp m, Ln(1+q)
accum -- one natural_log_exp table set -- + eb copy on the remaining
tiles).  DMA: dual hardware queues (sync=t+late-x, act=early-x)
measured 415 GB/s aggregate vs 347 single-queue; the ACT queue is
primed NIN_X tiles deep before compute, later x tiles ride the idle
sync queue.  A dummy activation at ACT start pulls the ~1.3us
ACT_TABLE_LOAD into the DMA fill window.

Sharding: batch dim 64 -> 8 per core, data parallel; host sums the
[128, NT] sp accumulator and the [1, 4096] wing row in f64.
"""

import sys
from contextlib import ExitStack

import numpy as np

sys.path.insert(0, "/opt/trn_rl_repo")

import concourse.bass as bass
import concourse.mybir as mybir
from concourse.bass_utils import run_bass_kernel_spmd

P = 128          # SBUF partitions
FREE = 4096      # slot capacity (max tile size)
FT = 16384       # elems per partition per core
NSLOT = 3        # rotating work-buffer slots
NIN_T = 3        # rotating t input slots (f32)
NIN_X = 4        # rotating x input slots (f32)
N_CORES = 8
B_SHARD = 8      # batches per core
CHUNK = 512      # psum bank width (f32)

# tile sizes: ramp-in/ramp-out shrink pipeline lead-in + drain
SIZES = [512, 2560, 4096, 4096, 3072, 2048]
assert sum(SIZES) == FT and all(s <= FREE for s in SIZES)
assert all(s % CHUNK == 0 for s in SIZES)
NT = len(SIZES)
OFFS = [sum(SIZES[:i]) for i in range(NT)]
EB_ON_DVE = (4, 5)  # tiles whose eb = 2.1-t runs on DVE (load balance)

NCHUNK = FREE // CHUNK
CHUNK_TILES = [[i for i in range(NT) if SIZES[i] > c * CHUNK] for c in range(NCHUNK)]

# weighted linear fit of h(e) = 2e/(1+2^e) on [1.1, 2.1], weight (3.2-2e)^2/8
C1 = 0.09109466425158937
C0 = 0.6200062494860638

ABS_MASK = 0x7FFF   # clears bf16 sign bit


def build_nc():
    dt = mybir.dt
    AF = mybir.ActivationFunctionType
    OP = mybir.AluOpType

    nc = bass.Bass()
    t_ext = nc.declare_dram_parameter("target", [P, FT], dt.float32, isOutput=False)
    x_ext = nc.declare_dram_parameter("input", [P, FT], dt.float32, isOutput=False)
    oacc = nc.declare_dram_parameter("out_acc", [P, NT], dt.float32, isOutput=True)
    ow = nc.declare_dram_parameter("out_w", [1, FREE], dt.float32, isOutput=True)

    t_sb = nc.alloc_sbuf_tensor("t_sb", [P, NIN_T * FREE], dt.float32).ap()
    x_sb = nc.alloc_sbuf_tensor("x_sb", [P, NIN_X * FREE], dt.float32).ap()
    b1 = nc.alloc_sbuf_tensor("b1_sb", [P, NSLOT * FREE], dt.bfloat16).ap()
    b2 = nc.alloc_sbuf_tensor("b2_sb", [P, NSLOT * FREE], dt.bfloat16).ap()
    b3 = nc.alloc_sbuf_tensor("b3_sb", [P, NSLOT * FREE], dt.bfloat16).ap()
    acc_sp = nc.alloc_sbuf_tensor("acc", [P, NT], dt.float32).ap()
    ones = nc.alloc_sbuf_tensor("ones_sb", [P, P], dt.bfloat16).ap()
    scr = nc.alloc_sbuf_tensor("scr_sb", [P, 1], dt.bfloat16).ap()
    wsum = nc.alloc_psum_tensor("wsum", [P, FREE], dt.float32).ap()
    wrow = nc.alloc_sbuf_tensor("wrow_sb", [1, FREE], dt.float32).ap()

    u16 = lambda ap: ap.bitcast(dt.uint16)

    def slot(buf, i):
        k = i % NSLOT
        return buf[:, k * FREE : k * FREE + SIZES[i]]

    def inslot_t(i):
        k = i % NIN_T
        return t_sb[:, k * FREE : k * FREE + SIZES[i]]

    def inslot_x(i):
        k = i % NIN_X
        return x_sb[:, k * FREE : k * FREE + SIZES[i]]

    def dsl(i):
        return slice(OFFS[i], OFFS[i] + SIZES[i])

    with ExitStack() as ctx:
        sem_t = [ctx.enter_context(nc.semaphore(f"t{i}")) for i in range(NT)]
        sem_x = [ctx.enter_context(nc.semaphore(f"x{i}")) for i in range(NT)]
        sDC = ctx.enter_context(nc.semaphore("sDC"))  # DVE: diff+dc done
        sAB = ctx.enter_context(nc.semaphore("sAB"))  # ACT: lnc (+eb) done
        sM = ctx.enter_context(nc.semaphore("sM"))    # DVE: m done
        sD = ctx.enter_context(nc.semaphore("sD"))    # ACT: sp accum done
        sWT = ctx.enter_context(nc.semaphore("sWT"))  # DVE: wing product done
        sPE = ctx.enter_context(nc.semaphore("sPE"))  # PE: wing reduce done
        sOne = ctx.enter_context(nc.semaphore("sOne"))
        sCP = ctx.enter_context(nc.semaphore("sCP"))
        s_out = ctx.enter_context(nc.semaphore("outdma"))
        block = ctx.enter_context(nc.Block())

        @block.sync
        def _(sync):
            for i in range(NT):
                if i >= NIN_T:
                    # t slot reuse: diff(i-NIN_T) (sDC) and eb(i-NIN_T) (sAB:
                    # lnc comes after eb in ACT program order; DVE eb precedes
                    # diff so sDC covers it there)
                    sync.wait_ge(sDC, i - NIN_T + 1)
                    sync.wait_ge(sAB, i - NIN_T + 1)
                sync.dma_start(inslot_t(i), t_ext[:, dsl(i)]).then_inc(sem_t[i], 16)
                if i >= NIN_T and i - NIN_T + NIN_X < NT:
                    # late x tiles ride the (idle) sync queue; slot hazard for
                    # x(j+NIN_X) needs diff(j) done: sDC wait above covers it
                    j = i - NIN_T + NIN_X
                    sync.dma_start(inslot_x(j), x_ext[:, dsl(j)]).then_inc(
                        sem_x[j], 16
                    )
            sync.wait_ge(sD, NT)
            sync.dma_start(oacc[:], acc_sp[:]).then_inc(s_out, 16)
            sync.wait_ge(s_out, 32)

        @block.vector
        def _(vector):
            nc.vector.memset(ones, 1.0).then_inc(sOne, 1)

            def stage_front(i):
                if i in EB_ON_DVE:
                    vector.wait_ge(sem_t[i], 16)
                    nc.vector.tensor_scalar(
                        slot(b3, i), inslot_t(i), -1.0, 2.1, OP.mult, OP.add
                    )
                vector.wait_ge(sem_t[i], 16)
                vector.wait_ge(sem_x[i], 16)
                if i >= NSLOT:
                    # b2 slot reuse: sp-accum(i-NSLOT); b1 reuse: PE(i-NSLOT)
                    vector.wait_ge(sD, i - NSLOT + 1)
                    vector.wait_ge(sPE, i - NSLOT + 1)
                nc.vector.tensor_tensor(
                    slot(b1, i), inslot_t(i), inslot_x(i), op=OP.subtract
                )
                nc.vector.tensor_scalar(
                    u16(slot(b1, i)), u16(slot(b1, i)), ABS_MASK, None, OP.bitwise_and
                )
                nc.vector.tensor_scalar(
                    slot(b2, i), slot(b1, i), 0.5, None, OP.min
                ).then_inc(sDC, 1)
                nc.vector.tensor_scalar(
                    slot(b1, i), slot(b1, i), -0.5, 0.0, OP.add, OP.max
                )

            def stage_tail(i):
                vector.wait_ge(sAB, i + 1)
                nc.vector.tensor_mul(slot(b2, i), slot(b3, i), slot(b2, i)).then_inc(
                    sM, 1
                )
                nc.vector.tensor_scalar(
                    slot(b3, i), slot(b3, i), C1, C0, OP.mult, OP.add
                )
                nc.vector.tensor_mul(slot(b1, i), slot(b3, i), slot(b1, i)).then_inc(
                    sWT, 1
                )

            for s in range(NT + 1):
                if s < NT:
                    stage_front(s)
                if 0 <= s - 1 < NT:
                    stage_tail(s - 1)
            vector.wait_ge(sPE, 5)
            nc.vector.tensor_copy(wrow[:, 2048:], wsum[0:1, 2048:])
            vector.wait_ge(sPE, NT)
            nc.vector.tensor_copy(wrow[:, :2048], wsum[0:1, :2048]).then_inc(sCP, 1)

        @block.scalar
        def _(scalar):
            # prime the x queue: first NIN_X tiles have no slot hazard
            for j in range(min(NIN_X, NT)):
                nc.scalar.dma_start(inslot_x(j), x_ext[:, dsl(j)]).then_inc(
                    sem_x[j], 16
                )
            # dummy activation: trigger the ACT table load during the fill
            scalar.wait_ge(sOne, 1)
            nc.scalar.activation(scr, ones[:, 0:1], AF.Ln)

            def stage_a(i):
                if i not in EB_ON_DVE:
                    scalar.wait_ge(sem_t[i], 16)  # t arrived
                    if i >= NSLOT:
                        # b3 slot reuse: wing product(i-NSLOT) consumed eb/h
                        scalar.wait_ge(sWT, i - NSLOT + 1)
                    nc.scalar.activation(
                        slot(b3, i), inslot_t(i), AF.Copy, scale=-1.0, bias=2.1
                    )
                scalar.wait_ge(sDC, i + 1)
                nc.scalar.activation(slot(b2, i), slot(b2, i), AF.Ln).then_inc(sAB, 1)

            def stage_f(i):
                scalar.wait_ge(sM, i + 1)
                nc.scalar.activation(slot(b2, i), slot(b2, i), AF.Exp)
                nc.scalar.activation(
                    slot(b2, i),
                    slot(b2, i),
                    AF.Ln,
                    bias=1.0,
                    accum_out=acc_sp[:, i : i + 1],
                ).then_inc(sD, 1)

            for s in range(NT + 1):
                if s < NT:
                    stage_a(s)
                if 0 <= s - 1 < NT:
                    stage_f(s - 1)
            scalar.wait_ge(sCP, 1)
            nc.scalar.dma_start(ow[:], wrow[:]).then_inc(s_out, 16)

        @block.tensor
        def _(tensor):
            tensor.wait_ge(sOne, 1)
            for i in range(NT):
                tensor.wait_ge(sWT, i + 1)
                nchunks = SIZES[i] // CHUNK
                wt = slot(b1, i)
                for c in range(nchunks):
                    mm = nc.tensor.matmul(
                        wsum[:, c * CHUNK : (c + 1) * CHUNK],
                        ones,
                        wt[:, c * CHUNK : (c + 1) * CHUNK],
                        start=(i == CHUNK_TILES[c][0]),
                        stop=(i == CHUNK_TILES[c][-1]),
                    )
                mm.then_inc(sPE, 1)

    return nc


_NC = None


def _get_nc():
    global _NC
    if _NC is None:
        _NC = build_nc()
    return _NC


def kernel(input, target, _trace=False, _nc=None):
    x = np.ascontiguousarray(np.asarray(input, dtype=np.float32))
    t = np.ascontiguousarray(np.asarray(target, dtype=np.float32))
    in_maps = []
    for i in range(N_CORES):
        bs = slice(i * B_SHARD, (i + 1) * B_SHARD)
        in_maps.append(
            {
                "input": x[bs].reshape(P, FT),
                "target": t[bs].reshape(P, FT),
            }
        )
    nc = _nc if _nc is not None else _get_nc()
    out = run_bass_kernel_spmd(nc, in_maps, core_ids=list(range(N_CORES)), trace=_trace)
    total = 0.0
    for res in out.results:
        total += res["out_acc"].astype(np.float64).sum()
        total += res["out_w"].astype(np.float64).sum()
    result = np.float32(14.0 * total)
    if _trace:
        return result, out
    return result
